# revision 62
# baseline (speedup 1.0000x reference)
"""Trainium2 Bass kernel for nn_MultiHeadAttention (B=2048, T=32, E=1024, H=16).

Sharding: data-parallel over batch, 256 batches per core x 8 cores.

v3 design (vs the PE-transpose v1): all transposes run on the DMA XBAR
(dma_start_transpose), x is fed as fp16 from host and y returned as fp16
(upconverted on host), the output bias is added during the Y evacuation
(tensor_tensor, no bias matmul), and the per-macro-tile work is software-
pipelined (projections of tile i issue ahead of attention of tile i-1) so
the PE fills attention-phase bubbles with projection matmuls.  PSUM banks
are partitioned per phase (3 proj / 1 Y / 2 scores / 2 attn-out) so pool
rotation never couples next-tile projections to the attention tail.  The
first X^T is split (128+384 rows) and interleaved with per-ht wq/wk DMA
chunks to shorten the startup ramp; the last macro-tile is tapered into
256+128+128-row segments to shorten the final attention/output drain.

Per-core pipeline (all matmuls fp16 with fp32 PSUM accumulation):
  XT  = X^T via DMA XBAR transpose straight from DRAM -> [128, et, bt]
  QT  = (Wq * HS^-0.5)^T @ ... = [hd, bt] (stationary = Wq chunk, moving = XT)
  KT  = same for Wk
  V'  = [bt, hd] with a ones column per head (softmax denominator trick)
  per (head-group, 4-batch group):
    S  = KT_slice.T @ QT_slice   -> [128,128] all cross-batch scores
    EB = exp(S) * maskEB         (block-diag causal mask kills cross terms)
    U' = EB.T @ V'_slice         -> [t, 64+1]; col 64 = softmax denominator
    O  = U'[:, :64] * 1/U'[:,64] (fused into PSUM evac copies)
  OT  = O^T via DMA XBAR transpose (SBUF -> SBUF)
  y   = OT.T @ Wp + bias         (bias added in the evac tensor_tensor)
"""
import sys
import numpy as np

sys.path.insert(0, "/opt/trn_rl_repo")

import concourse.bacc as bacc  # noqa: E402
import concourse.mybir as mybir  # noqa: E402
import concourse.tile as tile  # noqa: E402
from concourse.bass_utils import run_bass_kernel_spmd  # noqa: E402

B, T, E, H = 2048, 32, 1024, 16
HS = E // H            # 64
NCORES = 8
BC = B // NCORES       # 256 batches per core
BT = BC * T            # 8192 rows per core
P = 128
ET = E // P            # 8 e-tiles
MT = 512               # rows per macro-tile
NMT = BT // MT         # 16
NBT = MT // P          # 4 bt-tiles (= 4-batch groups) per macro-tile

F16 = mybir.dt.float16
F32 = mybir.dt.float32
AF = mybir.ActivationFunctionType

_CACHE = {}

HEAD_GROUPS = ((0, 2, 4, 6), (8, 10, 12, 14), (1, 3, 5, 7), (9, 11, 13, 15))


def _build_nc(nmt=NMT):
    bt = nmt * MT
    nc = bacc.Bacc(trn_type="TRN2")

    x_d = nc.dram_tensor("xs", [bt, E], F16, kind="ExternalInput")
    y_d = nc.dram_tensor("ys", [bt, E], F16, kind="ExternalOutput")
    # wq/wk layout: [p, ht, et, c] so each ht-chunk is a contiguous DMA
    wq_d = nc.dram_tensor("wq", [P, ET * E], F16, kind="ExternalInput")
    wk_d = nc.dram_tensor("wk", [P, ET * E], F16, kind="ExternalInput")
    wv_d = nc.dram_tensor("wv", [P, ET * E], F16, kind="ExternalInput")
    wp_d = nc.dram_tensor("wp", [P, ET * E], F16, kind="ExternalInput")
    bb_d = nc.dram_tensor("bb", [P, E], F32, kind="ExternalInput")
    mask_d = nc.dram_tensor("mask", [P, 512], F16, kind="ExternalInput")

    with tile.TileContext(nc) as tc:
        with (
            tc.tile_pool(name="const", bufs=1) as cpool,
            tc.tile_pool(name="xt", bufs=3) as xtpool,
            tc.tile_pool(name="qt", bufs=3) as qtpool,
            tc.tile_pool(name="kt", bufs=2) as ktpool,
            tc.tile_pool(name="vp", bufs=8) as vppool,
            tc.tile_pool(name="eb", bufs=8) as ebpool,
            tc.tile_pool(name="osb", bufs=8) as opool,
            tc.tile_pool(name="ot", bufs=8) as otpool,
            tc.tile_pool(name="rc", bufs=8) as rcpool,
            tc.tile_pool(name="yo", bufs=8) as ypool,
            tc.tile_pool(name="ps_mm", bufs=3, space="PSUM") as psmm,
            tc.tile_pool(name="ps_y", bufs=1, space="PSUM") as psy,
            tc.tile_pool(name="ps_s", bufs=2, space="PSUM") as pss,
            tc.tile_pool(name="ps_u", bufs=2, space="PSUM") as psu,
        ):
            wq = cpool.tile([P, ET * E], F16)
            wk = cpool.tile([P, ET * E], F16)
            wv = cpool.tile([P, ET * E], F16)
            wp = cpool.tile([P, ET * E], F16)
            bb = cpool.tile([P, E], F32)
            mask = cpool.tile([P, 512], F16)

            # Row segments (one per macro-tile; tail tapered so the final
            # attention/output drain covers less work).
            segs = [(i * MT, MT) for i in range(nmt - 1)]
            r0 = (nmt - 1) * MT
            for rows in (256, 128, 128):
                segs.append((r0, rows))
                r0 += rows
            nseg = len(segs)

            def xt_load(si, sub=None, tag_i=0):
                row0, rows = segs[si]
                if sub is not None:
                    row0 += sub[0]
                    rows = sub[1]
                t = xtpool.tile([P, ET, rows], F16, tag="xt",
                                name=f"xts{si}_{tag_i}")
                nc.sync.dma_start_transpose(t[:], x_d[row0:row0 + rows, :])
                return t

            # The first segment's X^T is split unevenly (128 + 384 rows) and
            # interleaved with the startup-critical wq chunks so the first QT
            # matmuls can begin as early as possible.
            nc.sync.dma_start(wq[:, 0:E], wq_d[:, 0:E])
            xts0a = xt_load(0, sub=(0, 128), tag_i=1)
            xts0b = xt_load(0, sub=(128, 384), tag_i=2)
            for ht in range(1, ET):
                nc.sync.dma_start(
                    wq[:, ht * E:(ht + 1) * E], wq_d[:, ht * E:(ht + 1) * E]
                )
            xts_q = [[(xts0a, 0, 128), (xts0b, 128, 384)]]
            if nseg > 1:
                xts_q.append(xt_load(1))
            for ht in range(ET):
                nc.sync.dma_start(
                    wk[:, ht * E:(ht + 1) * E], wk_d[:, ht * E:(ht + 1) * E]
                )
            if nseg > 2:
                xts_q.append(xt_load(2))
            nc.sync.dma_start(wv[:], wv_d[:])
            nc.sync.dma_start(wp[:], wp_d[:])
            nc.sync.dma_start(bb[:], bb_d[:])
            nc.sync.dma_start(mask[:], mask_d[:])

            def proj(si, xts):
                rows = segs[si][1]
                nb = rows // P
                if isinstance(xts, list):
                    parts = xts
                else:
                    parts = [(xts, 0, rows)]
                # ---- QT / KT projections: [hd, bt] ----
                qts = qtpool.tile([P, ET, rows], F16, tag="qt", name=f"qts{si}")
                kts = ktpool.tile([P, ET, rows], F16, tag="kt", name=f"kts{si}")
                for w_sb, dst in ((wq, qts), (wk, kts)):
                    for ht in range(ET):
                        for pt, col0, w in parts:
                            pq = psmm.tile([P, w], F32, tag="ps_mm")
                            for et in range(ET):
                                nc.tensor.matmul(
                                    pq[:],
                                    w_sb[:, ht * E + P * et:ht * E + P * (et + 1)],
                                    pt[:, et, :],
                                    start=(et == 0),
                                    stop=(et == ET - 1),
                                )
                            nc.vector.tensor_copy(
                                dst[:, ht, col0:col0 + w], pq[:]
                            )

                # ---- V projection -> V' [bt, 16*(64+1)] with ones cols ----
                vps = []
                for b in range(nb):
                    vp_t = vppool.tile([P, H * (HS + 1)], F16, tag="vp",
                                       name=f"vp{si}_{b}")
                    nc.vector.memset(
                        vp_t.rearrange("p (h c) -> p h c", c=HS + 1)[:, :, HS:HS + 1],
                        1.0,
                    )
                    for h2 in range(2):
                        pv = psmm.tile([P, 512], F32, tag="ps_mm")
                        pt, col0, _w = next(
                            p for p in parts
                            if p[1] <= P * b < p[1] + p[2]
                        )
                        for et in range(ET):
                            nc.tensor.matmul(
                                pv[:],
                                pt[:, et, P * b - col0:P * (b + 1) - col0],
                                wv[:, et * E + 512 * h2:et * E + 512 * (h2 + 1)],
                                start=(et == 0),
                                stop=(et == ET - 1),
                            )
                        dst = vp_t[:, 8 * (HS + 1) * h2:8 * (HS + 1) * (h2 + 1)]
                        nc.scalar.activation(
                            dst.rearrange("p (h c) -> p h c", c=HS + 1)[:, :, 0:HS],
                            pv.rearrange("p (h c) -> p h c", c=HS)[:],
                            AF.Copy,
                        )
                    vps.append(vp_t)
                return qts, kts, vps

            def attn_out(si, qts, kts, vps, last=False):
                row0 = segs[si][0]
                # ---- attention ----
                os_ = []
                for b in range(len(vps)):
                    o_t = opool.tile([P, E], F16, tag="o", name=f"o{si}_{b}")
                    os_.append(o_t)
                    for hq in range(4):
                        heads = HEAD_GROUPS[hq]
                        ps_s = pss.tile([P, 512], F32, tag="ps_s")
                        for hh in range(4):
                            h = heads[hh]
                            ht, hp = divmod(h, 2)
                            rs = slice(64 * hp, 64 * (hp + 1))
                            cs = slice(P * b, P * (b + 1))
                            nc.tensor.matmul(
                                ps_s[:, P * hh:P * (hh + 1)],
                                kts[rs, ht, cs],
                                qts[rs, ht, cs],
                                start=True,
                                stop=True,
                            )
                        ebt = ebpool.tile([P, 512], F16, tag="eb",
                                          name=f"eb{si}_{b}_{hq}")
                        nc.scalar.activation(ebt[:], ps_s[:], AF.Exp)
                        nc.gpsimd.tensor_mul(ebt[:], ebt[:], mask[:])
                        pu = psu.tile([P, 4 * (HS + 1)], F32, tag="ps_u")
                        for hh in range(4):
                            h = heads[hh]
                            nc.tensor.matmul(
                                pu[:, (HS + 1) * hh:(HS + 1) * (hh + 1)],
                                ebt[:, P * hh:P * (hh + 1)],
                                vps[b][:, (HS + 1) * h:(HS + 1) * (h + 1)],
                                start=True,
                                stop=True,
                            )
                        rc_t = rcpool.tile([P, 4], F32, tag="rc",
                                           name=f"rc{si}_{b}_{hq}")
                        nc.vector.reciprocal(
                            rc_t[:],
                            pu.rearrange("p (h c) -> p h c", c=HS + 1)[:, :, HS:HS + 1],
                        )
                        for hh in range(4):
                            h = heads[hh]
                            dst = o_t[:, HS * h:HS * (h + 1)]
                            src = pu[:, (HS + 1) * hh:(HS + 1) * hh + HS]
                            sc = rc_t[:, hh:hh + 1]
                            if hh == 0 and not last:
                                nc.scalar.activation(dst, src, AF.Copy, scale=sc)
                            else:
                                nc.vector.tensor_scalar_mul(dst, src, sc)

                # ---- O^T via DMA XBAR transpose + output projection ----
                for b in range(len(vps)):
                    if last:
                        ot_a = otpool.tile([P, ET // 2, P], F16, tag="ot",
                                           name=f"ota{si}_{b}")
                        ot_b = otpool.tile([P, ET // 2, P], F16, tag="ot",
                                           name=f"otb{si}_{b}")
                        nc.sync.dma_start_transpose(
                            ot_a[:], os_[b][:, 0:E // 2])
                        nc.sync.dma_start_transpose(
                            ot_b[:], os_[b][:, E // 2:E])

                        def ot_sl(ht):
                            return (ot_a[:, ht, :] if ht < ET // 2
                                    else ot_b[:, ht - ET // 2, :])
                    else:
                        ot_t = otpool.tile([P, ET, P], F16, tag="ot",
                                           name=f"ot{si}_{b}")
                        nc.sync.dma_start_transpose(ot_t[:], os_[b][:])

                        def ot_sl(ht):
                            return ot_t[:, ht, :]
                    y_t = ypool.tile([P, E], F16, tag="y", name=f"y{si}_{b}")
                    fin = last and b == len(vps) - 1
                    nq = 4 if fin else 2
                    qw = E // nq
                    for q in range(nq):
                        if last:
                            py = psmm.tile([P, qw], F32, tag="ps_mm")
                        else:
                            py = psy.tile([P, qw], F32, tag="ps_y")
                        for ht in range(ET):
                            nc.tensor.matmul(
                                py[:],
                                ot_sl(ht),
                                wp[:, ht * E + qw * q:ht * E + qw * (q + 1)],
                                start=(ht == 0),
                                stop=(ht == ET - 1),
                            )
                        nc.vector.tensor_add(
                            y_t[:, qw * q:qw * (q + 1)],
                            py[:],
                            bb[:, qw * q:qw * (q + 1)],
                        )
                        if fin:
                            nc.sync.dma_start(
                                y_d[row0 + P * b:row0 + P * (b + 1),
                                    qw * q:qw * (q + 1)],
                                y_t[:, qw * q:qw * (q + 1)],
                            )
                    if not fin:
                        nc.sync.dma_start(
                            y_d[row0 + P * b:row0 + P * (b + 1), :], y_t[:]
                        )

            # Software pipeline: projections of segment si are issued
            # (priority-wise) ahead of attention/output of si-1 so the PE can
            # fill attention-phase bubbles with projection matmuls.
            prev = None
            for si in range(nseg):
                if si + 3 <= nseg - 1 and len(xts_q) <= si + 3:
                    xts_q.append(xt_load(si + 3))
                cur = proj(si, xts_q[si])
                if prev is not None:
                    attn_out(si - 1, *prev)
                prev = cur
            attn_out(nseg - 1, *prev, last=True)

    nc.compile()
    return nc


def _host_prep(Wq, Wk, Wv, Wp, bp):
    def cat(w):  # [H, E, HS] -> [E, E]
        return np.ascontiguousarray(w.transpose(1, 0, 2).reshape(E, E))

    def sb_layout(w16):  # [E, E] f16 -> [128, 8*E], free = (et, col)
        return np.ascontiguousarray(
            w16.reshape(ET, P, E).transpose(1, 0, 2).reshape(P, ET * E)
        )

    def sb_layout_ht(w16):  # [E, E] f16 -> [128, 8*E], free = (ht, et, col)
        return np.ascontiguousarray(
            w16.reshape(ET, P, ET, P).transpose(1, 2, 0, 3).reshape(P, ET * E)
        )

    wq16 = sb_layout_ht((cat(Wq) * (HS ** -0.5)).astype(np.float16))
    wk16 = sb_layout_ht(cat(Wk).astype(np.float16))
    wv16 = sb_layout(cat(Wv).astype(np.float16))
    wp16 = sb_layout(Wp.astype(np.float16))
    bb = np.ascontiguousarray(
        np.broadcast_to(bp.astype(np.float32).reshape(1, E), (P, E))
    )

    m = np.zeros((P, P), dtype=np.float16)
    trilT = np.tril(np.ones((T, T))).T.astype(np.float16)  # [s,t], s<=t
    for i in range(4):
        m[T * i:T * (i + 1), T * i:T * (i + 1)] = trilT
    mask = np.ascontiguousarray(np.tile(m, (1, 4)))

    return dict(wq=wq16, wk=wk16, wv=wv16, wp=wp16, bb=bb, mask=mask)


def _run(x, Wq, Wk, Wv, Wp, bp, trace=False):
    if "nc" not in _CACHE:
        _CACHE["nc"] = _build_nc()
    nc = _CACHE["nc"]

    consts = _host_prep(
        np.asarray(Wq), np.asarray(Wk), np.asarray(Wv),
        np.asarray(Wp), np.asarray(bp),
    )
    x16 = np.asarray(x).astype(np.float16).reshape(NCORES, BT, E)
    in_maps = []
    for c in range(NCORES):
        in_maps.append({"xs": np.ascontiguousarray(x16[c]), **consts})

    res = run_bass_kernel_spmd(
        nc, in_maps, core_ids=list(range(NCORES)), trace=trace
    )
    y = np.concatenate(
        [res.results[c]["ys"].reshape(BC, T, E) for c in range(NCORES)], axis=0
    )
    return y.astype(np.float32), res


def kernel(x, Wq, Wk, Wv, Wp, bp):
    y, _ = _run(x, Wq, Wk, Wv, Wp, bp, trace=False)
    return y



# revision 67
# speedup vs baseline: 58.9402x; 58.9402x over previous
"""Trainium2 Bass kernel for nn_MultiHeadAttention (B=2048, T=32, E=1024, H=16).

Sharding: data-parallel over batch, 256 batches per core x 8 cores.

v3 design (vs the PE-transpose v1): all transposes run on the DMA XBAR
(dma_start_transpose), x is fed as fp16 from host and y returned as fp16
(upconverted on host), the output bias is added during the Y evacuation
(tensor_tensor, no bias matmul), and the per-macro-tile work is software-
pipelined (projections of tile i issue ahead of attention of tile i-1) so
the PE fills attention-phase bubbles with projection matmuls.  PSUM banks
are partitioned per phase (3 proj / 1 Y / 2 scores / 2 attn-out) so pool
rotation never couples next-tile projections to the attention tail.  The
first X^T is split (128+384 rows) and interleaved with per-ht wq/wk DMA
chunks to shorten the startup ramp; the last macro-tile is tapered into
256+128+128-row segments to shorten the final attention/output drain.

Per-core pipeline (all matmuls fp16 with fp32 PSUM accumulation):
  XT  = X^T via DMA XBAR transpose straight from DRAM -> [128, et, bt]
  QT  = (Wq * HS^-0.5)^T @ ... = [hd, bt] (stationary = Wq chunk, moving = XT)
  KT  = same for Wk
  V'  = [bt, hd] with a ones column per head (softmax denominator trick)
  per (head-group, 4-batch group):
    S  = KT_slice.T @ QT_slice   -> [128,128] all cross-batch scores
    EB = exp(S) * maskEB         (block-diag causal mask kills cross terms)
    U' = EB.T @ V'_slice         -> [t, 64+1]; col 64 = softmax denominator
    O  = U'[:, :64] * 1/U'[:,64] (fused into PSUM evac copies)
  OT  = O^T via DMA XBAR transpose (SBUF -> SBUF)
  y   = OT.T @ Wp + bias         (bias added in the evac tensor_tensor)
"""
import sys
import numpy as np

sys.path.insert(0, "/opt/trn_rl_repo")

import concourse.bacc as bacc  # noqa: E402
import concourse.mybir as mybir  # noqa: E402
import concourse.tile as tile  # noqa: E402
from concourse.bass_utils import run_bass_kernel_spmd  # noqa: E402

B, T, E, H = 2048, 32, 1024, 16
HS = E // H            # 64
NCORES = 8
BC = B // NCORES       # 256 batches per core
BT = BC * T            # 8192 rows per core
P = 128
ET = E // P            # 8 e-tiles
MT = 512               # rows per macro-tile
NMT = BT // MT         # 16
NBT = MT // P          # 4 bt-tiles (= 4-batch groups) per macro-tile

F16 = mybir.dt.float16
F32 = mybir.dt.float32
AF = mybir.ActivationFunctionType

_CACHE = {}

HEAD_GROUPS = ((0, 2, 4, 6), (8, 10, 12, 14), (1, 3, 5, 7), (9, 11, 13, 15))


def _build_nc(nmt=NMT):
    bt = nmt * MT
    nc = bacc.Bacc(trn_type="TRN2")

    x_d = nc.dram_tensor("xs", [bt, E], F16, kind="ExternalInput")
    y_d = nc.dram_tensor("ys", [bt, E], F16, kind="ExternalOutput")
    # wq/wk layout: [p, ht, et, c] so each ht-chunk is a contiguous DMA
    wq_d = nc.dram_tensor("wq", [P, ET * E], F16, kind="ExternalInput")
    wk_d = nc.dram_tensor("wk", [P, ET * E], F16, kind="ExternalInput")
    wv_d = nc.dram_tensor("wv", [P, ET * E], F16, kind="ExternalInput")
    wp_d = nc.dram_tensor("wp", [P, ET * E], F16, kind="ExternalInput")
    bb_d = nc.dram_tensor("bb", [P, E], F32, kind="ExternalInput")
    mask_d = nc.dram_tensor("mask", [P, 512], F16, kind="ExternalInput")

    with tile.TileContext(nc) as tc:
        with (
            tc.tile_pool(name="const", bufs=1) as cpool,
            tc.tile_pool(name="xt", bufs=3) as xtpool,
            tc.tile_pool(name="qt", bufs=3) as qtpool,
            tc.tile_pool(name="kt", bufs=2) as ktpool,
            tc.tile_pool(name="vp", bufs=8) as vppool,
            tc.tile_pool(name="eb", bufs=8) as ebpool,
            tc.tile_pool(name="osb", bufs=8) as opool,
            tc.tile_pool(name="ot", bufs=8) as otpool,
            tc.tile_pool(name="rc", bufs=8) as rcpool,
            tc.tile_pool(name="yo", bufs=8) as ypool,
            tc.tile_pool(name="ps_mm", bufs=3, space="PSUM") as psmm,
            tc.tile_pool(name="ps_y", bufs=1, space="PSUM") as psy,
            tc.tile_pool(name="ps_s", bufs=2, space="PSUM") as pss,
            tc.tile_pool(name="ps_u", bufs=2, space="PSUM") as psu,
        ):
            wq = cpool.tile([P, ET * E], F16)
            wk = cpool.tile([P, ET * E], F16)
            wv = cpool.tile([P, ET * E], F16)
            wp = cpool.tile([P, ET * E], F16)
            bb = cpool.tile([P, E], F32)
            mask = cpool.tile([P, 512], F16)

            # Row segments (one per macro-tile; tail tapered so the final
            # attention/output drain covers less work).
            segs = [(i * MT, MT) for i in range(max(nmt - 2, 0))]
            r0 = max(nmt - 2, 0) * MT
            taper = ((256, 256, 256, 128, 128) if nmt > 1
                     else (256, 128, 128))
            for rows in taper:
                segs.append((r0, rows))
                r0 += rows
            nseg = len(segs)

            def xt_load(si, sub=None, tag_i=0):
                row0, rows = segs[si]
                if sub is not None:
                    row0 += sub[0]
                    rows = sub[1]
                t = xtpool.tile([P, ET, rows], F16, tag="xt",
                                name=f"xts{si}_{tag_i}")
                nc.sync.dma_start_transpose(t[:], x_d[row0:row0 + rows, :])
                return t

            # The first segment's X^T is split unevenly (128 + 384 rows) and
            # interleaved with the startup-critical wq chunks so the first QT
            # matmuls can begin as early as possible.
            nc.sync.dma_start(wq[:, 0:E], wq_d[:, 0:E])
            xts0a = xt_load(0, sub=(0, 128), tag_i=1)
            xts0b = xt_load(0, sub=(128, segs[0][1] - 128), tag_i=2)
            for ht in range(1, ET):
                nc.sync.dma_start(
                    wq[:, ht * E:(ht + 1) * E], wq_d[:, ht * E:(ht + 1) * E]
                )
            xts_q = [[(xts0a, 0, 128), (xts0b, 128, segs[0][1] - 128)]]
            if nseg > 1:
                xts_q.append(xt_load(1))
            for ht in range(ET):
                nc.sync.dma_start(
                    wk[:, ht * E:(ht + 1) * E], wk_d[:, ht * E:(ht + 1) * E]
                )
            if nseg > 2:
                xts_q.append(xt_load(2))
            nc.sync.dma_start(wv[:], wv_d[:])
            nc.sync.dma_start(wp[:], wp_d[:])
            nc.sync.dma_start(bb[:], bb_d[:])
            nc.sync.dma_start(mask[:], mask_d[:])

            def proj(si, xts):
                rows = segs[si][1]
                nb = rows // P
                if isinstance(xts, list):
                    parts = xts
                else:
                    parts = [(xts, 0, rows)]
                # ---- QT / KT projections: [hd, bt] ----
                qts = qtpool.tile([P, ET, rows], F16, tag="qt", name=f"qts{si}")
                kts = ktpool.tile([P, ET, rows], F16, tag="kt", name=f"kts{si}")
                for w_sb, dst in ((wq, qts), (wk, kts)):
                    for ht in range(ET):
                        for pt, col0, w in parts:
                            pq = psmm.tile([P, w], F32, tag="ps_mm")
                            for et in range(ET):
                                nc.tensor.matmul(
                                    pq[:],
                                    w_sb[:, ht * E + P * et:ht * E + P * (et + 1)],
                                    pt[:, et, :],
                                    start=(et == 0),
                                    stop=(et == ET - 1),
                                )
                            nc.vector.tensor_copy(
                                dst[:, ht, col0:col0 + w], pq[:]
                            )

                # ---- V projection -> V' [bt, 16*(64+1)] with ones cols ----
                vps = []
                for b in range(nb):
                    vp_t = vppool.tile([P, H * (HS + 1)], F16, tag="vp",
                                       name=f"vp{si}_{b}")
                    nc.vector.memset(
                        vp_t.rearrange("p (h c) -> p h c", c=HS + 1)[:, :, HS:HS + 1],
                        1.0,
                    )
                    for h2 in range(2):
                        pv = psmm.tile([P, 512], F32, tag="ps_mm")
                        pt, col0, _w = next(
                            p for p in parts
                            if p[1] <= P * b < p[1] + p[2]
                        )
                        for et in range(ET):
                            nc.tensor.matmul(
                                pv[:],
                                pt[:, et, P * b - col0:P * (b + 1) - col0],
                                wv[:, et * E + 512 * h2:et * E + 512 * (h2 + 1)],
                                start=(et == 0),
                                stop=(et == ET - 1),
                            )
                        dst = vp_t[:, 8 * (HS + 1) * h2:8 * (HS + 1) * (h2 + 1)]
                        nc.scalar.activation(
                            dst.rearrange("p (h c) -> p h c", c=HS + 1)[:, :, 0:HS],
                            pv.rearrange("p (h c) -> p h c", c=HS)[:],
                            AF.Copy,
                        )
                    vps.append(vp_t)
                return qts, kts, vps

            def attn_out(si, qts, kts, vps, last=False):
                row0 = segs[si][0]
                # ---- attention ----
                os_ = []
                for b in range(len(vps)):
                    o_t = opool.tile([P, E], F16, tag="o", name=f"o{si}_{b}")
                    os_.append(o_t)
                    for hq in range(4):
                        heads = HEAD_GROUPS[hq]
                        ps_s = pss.tile([P, 512], F32, tag="ps_s")
                        for hh in range(4):
                            h = heads[hh]
                            ht, hp = divmod(h, 2)
                            rs = slice(64 * hp, 64 * (hp + 1))
                            cs = slice(P * b, P * (b + 1))
                            nc.tensor.matmul(
                                ps_s[:, P * hh:P * (hh + 1)],
                                kts[rs, ht, cs],
                                qts[rs, ht, cs],
                                start=True,
                                stop=True,
                            )
                        ebt = ebpool.tile([P, 512], F16, tag="eb",
                                          name=f"eb{si}_{b}_{hq}")
                        nc.scalar.activation(ebt[:], ps_s[:], AF.Exp)
                        nc.gpsimd.tensor_mul(ebt[:], ebt[:], mask[:])
                        pu = psu.tile([P, 4 * (HS + 1)], F32, tag="ps_u")
                        for hh in range(4):
                            h = heads[hh]
                            nc.tensor.matmul(
                                pu[:, (HS + 1) * hh:(HS + 1) * (hh + 1)],
                                ebt[:, P * hh:P * (hh + 1)],
                                vps[b][:, (HS + 1) * h:(HS + 1) * (h + 1)],
                                start=True,
                                stop=True,
                            )
                        rc_t = rcpool.tile([P, 4], F32, tag="rc",
                                           name=f"rc{si}_{b}_{hq}")
                        nc.vector.reciprocal(
                            rc_t[:],
                            pu.rearrange("p (h c) -> p h c", c=HS + 1)[:, :, HS:HS + 1],
                        )
                        for hh in range(4):
                            h = heads[hh]
                            dst = o_t[:, HS * h:HS * (h + 1)]
                            src = pu[:, (HS + 1) * hh:(HS + 1) * hh + HS]
                            sc = rc_t[:, hh:hh + 1]
                            if hh == 0 and not last:
                                nc.scalar.activation(dst, src, AF.Copy, scale=sc)
                            else:
                                nc.vector.tensor_scalar_mul(dst, src, sc)

                # ---- O^T via DMA XBAR transpose + output projection ----
                for b in range(len(vps)):
                    if last:
                        ot_a = otpool.tile([P, ET // 2, P], F16, tag="ot",
                                           name=f"ota{si}_{b}")
                        ot_b = otpool.tile([P, ET // 2, P], F16, tag="ot",
                                           name=f"otb{si}_{b}")
                        nc.sync.dma_start_transpose(
                            ot_a[:], os_[b][:, 0:E // 2])
                        nc.sync.dma_start_transpose(
                            ot_b[:], os_[b][:, E // 2:E])

                        def ot_sl(ht):
                            return (ot_a[:, ht, :] if ht < ET // 2
                                    else ot_b[:, ht - ET // 2, :])
                    else:
                        ot_t = otpool.tile([P, ET, P], F16, tag="ot",
                                           name=f"ot{si}_{b}")
                        nc.sync.dma_start_transpose(ot_t[:], os_[b][:])

                        def ot_sl(ht):
                            return ot_t[:, ht, :]
                    y_t = ypool.tile([P, E], F16, tag="y", name=f"y{si}_{b}")
                    fin = last and b == len(vps) - 1
                    nq = 4 if fin else 2
                    qw = E // nq
                    for q in range(nq):
                        if last:
                            py = psmm.tile([P, qw], F32, tag="ps_mm")
                        else:
                            py = psy.tile([P, qw], F32, tag="ps_y")
                        for ht in range(ET):
                            nc.tensor.matmul(
                                py[:],
                                ot_sl(ht),
                                wp[:, ht * E + qw * q:ht * E + qw * (q + 1)],
                                start=(ht == 0),
                                stop=(ht == ET - 1),
                            )
                        nc.vector.tensor_add(
                            y_t[:, qw * q:qw * (q + 1)],
                            py[:],
                            bb[:, qw * q:qw * (q + 1)],
                        )
                        if fin:
                            nc.sync.dma_start(
                                y_d[row0 + P * b:row0 + P * (b + 1),
                                    qw * q:qw * (q + 1)],
                                y_t[:, qw * q:qw * (q + 1)],
                            )
                    if not fin:
                        nc.sync.dma_start(
                            y_d[row0 + P * b:row0 + P * (b + 1), :], y_t[:]
                        )

            # Software pipeline: projections of segment si are issued
            # (priority-wise) ahead of attention/output of si-1 so the PE can
            # fill attention-phase bubbles with projection matmuls.
            prev = None
            for si in range(nseg):
                if si + 3 <= nseg - 1 and len(xts_q) <= si + 3:
                    xts_q.append(xt_load(si + 3))
                cur = proj(si, xts_q[si])
                if prev is not None:
                    attn_out(si - 1, *prev)
                prev = cur
            attn_out(nseg - 1, *prev, last=True)

    nc.compile()
    return nc


def _host_prep(Wq, Wk, Wv, Wp, bp):
    def cat(w):  # [H, E, HS] -> [E, E]
        return np.ascontiguousarray(w.transpose(1, 0, 2).reshape(E, E))

    def sb_layout(w16):  # [E, E] f16 -> [128, 8*E], free = (et, col)
        return np.ascontiguousarray(
            w16.reshape(ET, P, E).transpose(1, 0, 2).reshape(P, ET * E)
        )

    def sb_layout_ht(w16):  # [E, E] f16 -> [128, 8*E], free = (ht, et, col)
        return np.ascontiguousarray(
            w16.reshape(ET, P, ET, P).transpose(1, 2, 0, 3).reshape(P, ET * E)
        )

    wq16 = sb_layout_ht((cat(Wq) * (HS ** -0.5)).astype(np.float16))
    wk16 = sb_layout_ht(cat(Wk).astype(np.float16))
    wv16 = sb_layout(cat(Wv).astype(np.float16))
    wp16 = sb_layout(Wp.astype(np.float16))
    bb = np.ascontiguousarray(
        np.broadcast_to(bp.astype(np.float32).reshape(1, E), (P, E))
    )

    m = np.zeros((P, P), dtype=np.float16)
    trilT = np.tril(np.ones((T, T))).T.astype(np.float16)  # [s,t], s<=t
    for i in range(4):
        m[T * i:T * (i + 1), T * i:T * (i + 1)] = trilT
    mask = np.ascontiguousarray(np.tile(m, (1, 4)))

    return dict(wq=wq16, wk=wk16, wv=wv16, wp=wp16, bb=bb, mask=mask)


def _run(x, Wq, Wk, Wv, Wp, bp, trace=False):
    if "nc" not in _CACHE:
        _CACHE["nc"] = _build_nc()
    nc = _CACHE["nc"]

    consts = _host_prep(
        np.asarray(Wq), np.asarray(Wk), np.asarray(Wv),
        np.asarray(Wp), np.asarray(bp),
    )
    x16 = np.asarray(x).astype(np.float16).reshape(NCORES, BT, E)
    in_maps = []
    for c in range(NCORES):
        in_maps.append({"xs": np.ascontiguousarray(x16[c]), **consts})

    res = run_bass_kernel_spmd(
        nc, in_maps, core_ids=list(range(NCORES)), trace=trace
    )
    y = np.concatenate(
        [res.results[c]["ys"].reshape(BC, T, E) for c in range(NCORES)], axis=0
    )
    return y.astype(np.float32), res


def kernel(x, Wq, Wk, Wv, Wp, bp):
    y, _ = _run(x, Wq, Wk, Wv, Wp, bp, trace=False)
    return y

